# revision 34
# baseline (speedup 1.0000x reference)
"""GroupPretrainHead on 8 NeuronCores (Trainium2, Bass/Tile).

Expert-parallel sharding: core g owns group g's decoder (W[g], b[g]) and
processes exactly the samples routed to group g. The host does the routing
permutation (gather/scatter of rows = the MoE dispatch/combine step); the
device does all FLOPs: out.T = W[g] @ h.T + b[g] as a K-accumulated matmul.

Design (distilled from perfetto traces of 8 prior variants):
 - h/W/out in bf16 (PSUM accumulation stays fp32), bias f32.
 - hT streamed one k-tile per DMA, alternating the two HWDGE rings
   (sync/scalar) so tiles arrive in consumption order; the first six are
   pre-issued ahead of w because the ramp is trigger-issue-limited
   (~0.75us per DMA_DIRECT2D on the issuing engine). The last four
   k-tiles are split column-wise across BOTH rings so the stream does not
   end with lonely single-ring transfers. Big multi-k-tile chunks
   regress: per-engine packet FIFOs stretch a leading chunk's completion
   to near stream end, starving the PE.
 - PE pre-warm (24 throwaway matmuls on a memset tile) plus 1-2 filler
   matmuls per k-tile keep PE duty high enough that the HAM clock gate
   lifts to 2.4 GHz and STAYS lifted; at ~55% duty it re-throttles
   mid-kernel and the tail matmuls run at 1.2 GHz.
 - bias rides a SWDGE lane; w leads the sync ring. Throwaway absorber ops
   on PE/DVE/ACT soak the w/bias DMA semaphores so every real instruction
   needs at most ONE encoded wait (this walrus rejects more).
 - output packed [128, C2]: n-chunk pairs stacked two-deep in partitions
   (PE writes the odd chunk at PSUM partition offset 64). PSUM is split
   column-wise over two banks so VectorE and ScalarE evacuate in
   parallel, then one full-width bf16 HWDGE writeback from the scalar
   engine (its provably-redundant lane-reuse wait is stripped in IR).
 - kernel tail: drop the redundant semaphore clear + second barrier (the
   framework epilogue zeroes every semaphore right after anyway), use the
   sem-only rendezvous, and run the out-gated drain last.

Device-side layout per core (C = max group count, rounded up to 16):
  hT    [16, 128, C]  bf16 -- gathered hidden rows, transposed, k-major
  wT    [128, 16*64]  bf16 -- W[g] transposed to [d-partition, (ktile j)]
  bias2 [128, 1]      f32  -- b[g] stacked twice
  outT  [128, C2]     bf16 -- preds.T, chunk pairs at partitions 0/64
"""

import numpy as np
import ml_dtypes

N_GROUPS = 8
D_MODEL = 2048
MAX_GS = 64
PART = 128
KT = D_MODEL // PART  # 16
NCH = 512  # matmul n-chunk (one PSUM bank of f32)

TRACE = False
LAST_EXEC_NS = None
LAST_RESULTS = None

_nc_cache = {}


def _chunks(C):
    """n-chunk (offset, size) pairs and the packed outT column count."""
    offs = [(o, min(NCH, C - o)) for o in range(0, C, NCH)]
    c2 = 0
    for i in range(0, len(offs), 2):
        c2 += offs[i][1]
    return offs, c2


def _make_tile_context_cls():
    import concourse.mybir as mybir
    from concourse.tile import TileContext
    from concourse.vector_clock import ScopedClock

    class SplitDrainTileContext(TileContext):
        """This container's walrus encodes at most ONE semaphore wait per
        instruction; Tile's kernel-tail drain aggregates every outstanding
        sem onto a single InstDrain, which fails codegen. Split it into a
        chain of one-wait drains. Also skip the per-kernel semaphore clear
        and the second barrier: the framework epilogue zeroes all 256
        semaphores right after this block in every NEFF iteration."""

        def _drain_and_barrier(self, tick_clock, wait_clock):
            drain_inst = self.nc.sync.drain()
            wait_clock.add_sem_waits(
                drain_inst.ins, ScopedClock({None: tick_clock.global_clock})
            )
            si = drain_inst.ins.sync_info
            waits = list(si.on_wait) if si else []
            if len(waits) > 1:
                si.on_wait = waits[:1]
                drain_inst.ins.sync_info = si
                for w in waits[1:]:
                    d2 = self.nc.sync.drain()
                    d2.ins.sync_info = mybir.SyncInfo(on_wait=[w], on_update=[])
            # sem_only: plain-semaphore rendezvous (~0.3us) instead of the
            # event-semaphore chain (~2us of polling latency).
            self.nc.all_engine_barrier(sem_only=True)
            popped = self.nc._tile_sem_poison_stack.pop()
            assert popped is self._sem_poison

    return SplitDrainTileContext


def _build_nc(C):
    import concourse.bass as bass
    import concourse.mybir as mybir

    TileContext = _make_tile_context_cls()

    f32 = mybir.dt.float32
    bf16 = mybir.dt.bfloat16
    nc = bass.Bass()

    offs, C2 = _chunks(C)
    KTM = KT * MAX_GS

    hT = nc.declare_dram_parameter("hT", [KT, PART, C], bf16, isOutput=False)
    wT = nc.declare_dram_parameter("wT", [PART, KTM], bf16, isOutput=False)
    bias2 = nc.declare_dram_parameter("bias2", [PART, 1], f32, isOutput=False)
    outT = nc.declare_dram_parameter("outT", [PART, C2], bf16, isOutput=True)

    # The two HWDGE rings: SP (sync) and ACT (scalar). A third (SWDGE)
    # stream adds no bandwidth -- the HBM stack is already saturated at
    # ~330 GB/s/core with both stack neighbors streaming.
    def ring(i):
        return nc.sync if i % 2 == 0 else nc.scalar

    with TileContext(nc) as tc:
        with (
            tc.tile_pool(name="const", bufs=1) as constp,
            tc.tile_pool(name="h", bufs=KT) as hp,
            tc.tile_pool(name="psum", bufs=1, space=bass.MemorySpace.PSUM) as pp,
            tc.tile_pool(name="out", bufs=1) as op,
        ):
            # Pre-issue the first four k-tile DMAs BEFORE w: per-ring
            # trigger issue costs ~0.8us each, and the h-stream ramp is
            # trigger-limited for its first ~2us. w can land late -- the PE
            # pre-warm keeps the array busy until well after it arrives.
            h_tiles = {}
            for t in range(6):
                h_tiles[t] = hp.tile([PART, C], bf16, tag="h", name=f"h{t}")
                ring(t).dma_start(h_tiles[t][:], hT[t])
            w_sb = constp.tile([PART, KTM], bf16, tag="w")
            nc.sync.dma_start(w_sb[:], wT[:])
            b_sb = constp.tile([PART, 1], f32, tag="bias")
            nc.gpsimd.dma_start(b_sb[:], bias2[:])

            # PSUM layout: chunks 0/1 stacked two-deep in partitions (PE
            # writes chunk1 at partition offset 64), and SPLIT column-wise
            # over two banks so the final evacuation can run on VectorE and
            # ScalarE in parallel (same-bank parallel PSUM access is
            # serialized by the hardware).
            assert len(offs) == 3 and offs[0][1] == NCH and offs[1][1] == NCH
            SPL = 320
            rem = offs[2][1]
            psA1 = pp.tile([PART, SPL], f32, tag="psA1", name="psA1")
            psA2 = pp.tile([PART, NCH - SPL], f32, tag="psA2", name="psA2")
            psB = pp.tile([MAX_GS, rem], f32, tag="psB", name="psB")

            def targets(n):
                """(psum AP, chunk-relative col range) pairs for chunk n."""
                if n == 0:
                    return [(psA1[0:MAX_GS, :], 0, SPL), (psA2[0:MAX_GS, :], SPL, NCH)]
                if n == 1:
                    return [(psA1[MAX_GS:PART, :], 0, SPL), (psA2[MAX_GS:PART, :], SPL, NCH)]
                return [(psB[:, :], 0, rem)]

            # PE pre-warm: throwaway matmuls on a zeroed tile, issued before
            # any DMA dependency, keep the PE array busy from kernel start
            # so HAM lifts the clock gate (1.2 -> 2.4 GHz). The busy burst
            # must exceed the free-running 3413ns HAM window: 14 x 232ns =
            # 3.2us measurably does NOT fire it; use 24 (~5.6us).
            dw = constp.tile([PART, 256], bf16, tag="dwarm")
            nc.vector.memset(dw[:], 0)
            ps_warm = pp.tile([MAX_GS, 256], f32, tag="pswarm", name="pswarm")
            for _ in range(24):
                nc.tensor.matmul(
                    ps_warm[:, :], dw[:, 0:MAX_GS], dw[:, :],
                    start=True, stop=True,
                )

            # Absorb the w/bias DMA waits into throwaway ops on PE and DVE
            # so each real matmul waits only on its h-tile DMA and the first
            # tensor_scalar_add waits only on PE (one-wait-per-inst limit).
            nc.tensor.matmul(
                ps_warm[:, 0:MAX_GS], w_sb[:, 0:MAX_GS], w_sb[:, 0:MAX_GS],
                start=True, stop=True,
            )
            b_warm = constp.tile([PART, 1], f32, tag="bwarm", name="bwarm")
            nc.vector.tensor_copy(b_warm[:], b_sb[:])

            # k-tiles 0..13: one DMA each, alternating rings. k-tiles 14/15
            # are split column-wise across BOTH rings so the stream does not
            # end with two lonely ~160 GB/s single-ring transfers.
            for t in range(KT - 4):
                if t not in h_tiles:
                    h_tiles[t] = hp.tile([PART, C], bf16, tag="h", name=f"h{t}")
                    ring(t).dma_start(h_tiles[t][:], hT[t])
                h_sb = h_tiles[t]
                for n in range(len(offs)):
                    no = offs[n][0]
                    for ps_ap, c0, c1 in targets(n):
                        nc.tensor.matmul(
                            ps_ap,
                            w_sb[:, t * MAX_GS : (t + 1) * MAX_GS],
                            h_sb[:, no + c0 : no + c1],
                            start=(t == 0),
                            stop=False,
                        )
                if t >= 2:
                    # filler matmuls per k-tile keep PE duty high enough
                    # that HAM holds the 2.4 GHz clock through the stream;
                    # any density gap re-throttles within ~3.4us and the
                    # cold backlog pushes the finish past stream end.
                    for _ in range(2):
                        nc.tensor.matmul(
                            ps_warm[:, :], dw[:, 0:MAX_GS], dw[:, :],
                            start=True, stop=True,
                        )
            for t in range(KT - 4, KT):
                ha = hp.tile([PART, NCH], bf16, tag=f"ha{t}")
                nc.sync.dma_start(ha[:], hT[t, :, 0:NCH])
                hb = hp.tile([PART, C - NCH], bf16, tag=f"hb{t}")
                nc.scalar.dma_start(hb[:], hT[t, :, NCH:C])
                last = t == KT - 1
                w_ap = w_sb[:, t * MAX_GS : (t + 1) * MAX_GS]
                for ps_ap, c0, c1 in targets(0):
                    nc.tensor.matmul(ps_ap, w_ap, ha[:, c0:c1],
                                     start=False, stop=last)
                for ps_ap, c0, c1 in targets(1):
                    nc.tensor.matmul(ps_ap, w_ap, hb[:, c0:c1],
                                     start=False, stop=last)
                for ps_ap, c0, c1 in targets(2):
                    nc.tensor.matmul(ps_ap, w_ap, hb[:, NCH + c0 : NCH + c1],
                                     start=False, stop=last)
                if not last:
                    for _ in range(2):
                        nc.tensor.matmul(
                            ps_warm[:, :], dw[:, 0:MAX_GS], dw[:, :],
                            start=True, stop=True,
                        )

            # ScalarE-side bias absorber, emitted AFTER every scalar-ring
            # trigger so it can't block the h-stream while waiting on the
            # bias DMA.
            b_warm2 = constp.tile([PART, 1], f32, tag="bwarm2", name="bwarm2")
            nc.scalar.copy(b_warm2[:], b_sb[:])

            # Evacuate PSUM in parallel: DVE takes bank psA1, ACT takes
            # banks psA2 + psB. The out DMA is issued by the scalar engine,
            # so the ACT writes are ordered before it implicitly and its
            # single semaphore wait covers the DVE write.
            o_sb = op.tile([PART, C2], bf16, tag="o")
            nc.vector.memset(o_sb[MAX_GS:PART, NCH:C2], 0)
            nc.vector.tensor_scalar_add(o_sb[:, 0:SPL], psA1[:, :], b_sb[:])
            nc.scalar.add(o_sb[:, SPL:NCH], psA2[:, :], b_sb[:])
            nc.scalar.add(o_sb[0:MAX_GS, NCH:C2], psB[:, :], b_sb[0:MAX_GS])
            out_dma = nc.scalar.dma_start(outT[:], o_sb[:])

    # The out DMA reuses a DMAHW lane, so Tile gives it a lane-reuse wait
    # on top of its DVE data wait -- two waits, which this walrus rejects.
    # The lane's previous DMA is an h-tile transfer that every matmul (and
    # hence the DVE adds) already transitively ordered before us, so the
    # lane wait is provably satisfied: strip it, keep the DVE wait.
    si = out_dma.ins.sync_info
    if si is not None and len(si.on_wait) > 1:
        keep = [w for w in si.on_wait if "DVE" in (w.ant_name or "")]
        assert len(keep) == 1, [str(w) for w in si.on_wait]
        si.on_wait = keep
        out_dma.ins.sync_info = si

    # Reorder the kernel-tail drain chain so the single drain that waits on
    # the out DMA's completion sem runs LAST: the other lane waits are
    # satisfied mid-stream, and executing them first means the sync engine
    # takes its one slow semaphore wake-up on the final drain only.
    out_sem = out_dma.ins.sync_info.on_update[0].id
    drains = []
    for f in nc.m.functions:
        for blk in f.blocks:
            for inst in blk.instructions:
                if type(inst).__name__ == "InstDrain":
                    si2 = inst.sync_info
                    if si2 is not None and len(si2.on_wait) == 1:
                        drains.append(inst)
    hit = [d for d in drains if d.sync_info.on_wait[0].id == out_sem]
    if hit and drains and hit[0] is not drains[-1]:
        d_out, d_last = hit[0], drains[-1]
        si_a, si_b = d_out.sync_info, d_last.sync_info
        d_out.sync_info, d_last.sync_info = si_b, si_a

    return nc


def kernel(**inputs):
    global LAST_EXEC_NS, LAST_RESULTS
    from concourse.bass_utils import run_bass_kernel_spmd

    hidden = np.ascontiguousarray(np.asarray(inputs["hidden"], dtype=np.float32))
    idx = np.asarray(inputs["chosen_group_idx"]).astype(np.int64)
    W = np.asarray(inputs["W"], dtype=np.float32)
    b = np.asarray(inputs["b"], dtype=np.float32)
    gs = np.asarray(inputs["group_sizes"])

    B = hidden.shape[0]
    counts = np.bincount(idx, minlength=N_GROUPS)
    C = max(1040, int(-(-counts.max() // 16)) * 16)
    offs, C2 = _chunks(C)

    positions = [np.nonzero(idx == g)[0] for g in range(N_GROUPS)]

    bf = ml_dtypes.bfloat16
    in_maps = []
    for g in range(N_GROUPS):
        pos = positions[g]
        hg = np.zeros((C, D_MODEL), bf)
        hg[: len(pos)] = hidden[pos, g, :].astype(bf)
        hT = np.ascontiguousarray(hg.T).reshape(KT, PART, C)
        wT = np.ascontiguousarray(
            W[g].reshape(MAX_GS, KT, PART).transpose(2, 1, 0)
        ).reshape(PART, KT * MAX_GS).astype(bf)
        bias2 = np.ascontiguousarray(
            np.concatenate([b[g], b[g]])[:, None].astype(np.float32)
        )
        in_maps.append({"hT": hT, "wT": wT, "bias2": bias2})

    if C not in _nc_cache:
        _nc_cache[C] = _build_nc(C)
    nc = _nc_cache[C]

    res = run_bass_kernel_spmd(nc, in_maps, list(range(N_GROUPS)), trace=TRACE)
    LAST_EXEC_NS = res.exec_time_ns
    LAST_RESULTS = res

    preds = np.zeros((B, MAX_GS), np.float32)
    for g in range(N_GROUPS):
        pos = positions[g]
        outT = np.asarray(res.results[g]["outT"]).astype(np.float32)  # [128, C2]
        og = np.zeros((C, MAX_GS), np.float32)
        col = 0
        for i in range(0, len(offs), 2):
            no, ns = offs[i]
            og[no : no + ns] = outT[0:MAX_GS, col : col + ns].T
            if i + 1 < len(offs):
                no1, ns1 = offs[i + 1]
                og[no1 : no1 + ns1] = outT[MAX_GS:PART, col : col + ns1].T
            col += ns
        preds[pos] = og[: len(pos)]

    valid = np.arange(MAX_GS)[None, :] < gs[idx][:, None]
    preds = np.where(valid, preds, np.float32(0.0))
    return preds, valid


# revision 35
# speedup vs baseline: 1.0665x; 1.0665x over previous
"""GroupPretrainHead on 8 NeuronCores (Trainium2, Bass/Tile).

Expert-parallel sharding: core g owns group g's decoder (W[g], b[g]) and
processes exactly the samples routed to group g. The host does the routing
permutation (gather/scatter of rows = the MoE dispatch/combine step); the
device does all FLOPs: out.T = W[g] @ h.T + b[g] as a K-accumulated matmul.

Design (distilled from perfetto traces of 8 prior variants):
 - h/W/out in bf16 (PSUM accumulation stays fp32), bias f32.
 - hT streamed one k-tile per DMA, alternating the two HWDGE rings
   (sync/scalar) so tiles arrive in consumption order; the first six are
   pre-issued ahead of w because the ramp is trigger-issue-limited
   (~0.75us per DMA_DIRECT2D on the issuing engine). The last four
   k-tiles are split column-wise across BOTH rings so the stream does not
   end with lonely single-ring transfers. Big multi-k-tile chunks
   regress: per-engine packet FIFOs stretch a leading chunk's completion
   to near stream end, starving the PE.
 - PE pre-warm (24 throwaway matmuls on a memset tile) plus 1-2 filler
   matmuls per k-tile keep PE duty high enough that the HAM clock gate
   lifts to 2.4 GHz and STAYS lifted; at ~55% duty it re-throttles
   mid-kernel and the tail matmuls run at 1.2 GHz.
 - bias rides a SWDGE lane; w leads the sync ring. Throwaway absorber ops
   on PE/DVE/ACT soak the w/bias DMA semaphores so every real instruction
   needs at most ONE encoded wait (this walrus rejects more).
 - output packed [128, C2]: n-chunk pairs stacked two-deep in partitions
   (PE writes the odd chunk at PSUM partition offset 64). PSUM is split
   column-wise over two banks so VectorE and ScalarE evacuate in
   parallel, then one full-width bf16 HWDGE writeback from the scalar
   engine (its provably-redundant lane-reuse wait is stripped in IR).
 - kernel tail: drop the redundant semaphore clear + second barrier (the
   framework epilogue zeroes every semaphore right after anyway), use the
   sem-only rendezvous, and run the out-gated drain last.

Device-side layout per core (C = max group count, rounded up to 16):
  hT    [16, 128, C]  bf16 -- gathered hidden rows, transposed, k-major
  wT    [128, 16*64]  bf16 -- W[g] transposed to [d-partition, (ktile j)]
  bias2 [128, 1]      f32  -- b[g] stacked twice
  outT  [128, C2]     bf16 -- preds.T, chunk pairs at partitions 0/64
"""

import numpy as np
import ml_dtypes

N_GROUPS = 8
D_MODEL = 2048
MAX_GS = 64
PART = 128
KT = D_MODEL // PART  # 16
NCH = 512  # matmul n-chunk (one PSUM bank of f32)

TRACE = False
LAST_EXEC_NS = None
LAST_RESULTS = None

_nc_cache = {}


def _chunks(C):
    """n-chunk (offset, size) pairs and the packed outT column count."""
    offs = [(o, min(NCH, C - o)) for o in range(0, C, NCH)]
    c2 = 0
    for i in range(0, len(offs), 2):
        c2 += offs[i][1]
    return offs, c2


def _make_tile_context_cls():
    import concourse.mybir as mybir
    from concourse.tile import TileContext
    from concourse.vector_clock import ScopedClock

    class SplitDrainTileContext(TileContext):
        """This container's walrus encodes at most ONE semaphore wait per
        instruction; Tile's kernel-tail drain aggregates every outstanding
        sem onto a single InstDrain, which fails codegen. Split it into a
        chain of one-wait drains. Also skip the per-kernel semaphore clear
        and the second barrier: the framework epilogue zeroes all 256
        semaphores right after this block in every NEFF iteration."""

        def _drain_and_barrier(self, tick_clock, wait_clock):
            drain_inst = self.nc.sync.drain()
            wait_clock.add_sem_waits(
                drain_inst.ins, ScopedClock({None: tick_clock.global_clock})
            )
            si = drain_inst.ins.sync_info
            waits = list(si.on_wait) if si else []
            if len(waits) > 1:
                si.on_wait = waits[:1]
                drain_inst.ins.sync_info = si
                for w in waits[1:]:
                    d2 = self.nc.sync.drain()
                    d2.ins.sync_info = mybir.SyncInfo(on_wait=[w], on_update=[])
            # sem_only: plain-semaphore rendezvous (~0.3us) instead of the
            # event-semaphore chain (~2us of polling latency).
            self.nc.all_engine_barrier(sem_only=True)
            popped = self.nc._tile_sem_poison_stack.pop()
            assert popped is self._sem_poison

    return SplitDrainTileContext


def _build_nc(C):
    import concourse.bass as bass
    import concourse.mybir as mybir

    TileContext = _make_tile_context_cls()

    f32 = mybir.dt.float32
    bf16 = mybir.dt.bfloat16
    nc = bass.Bass()

    offs, C2 = _chunks(C)
    KTM = KT * MAX_GS

    hT = nc.declare_dram_parameter("hT", [KT, PART, C], bf16, isOutput=False)
    wT = nc.declare_dram_parameter("wT", [PART, KTM], bf16, isOutput=False)
    bias2 = nc.declare_dram_parameter("bias2", [PART, 1], f32, isOutput=False)
    outT = nc.declare_dram_parameter("outT", [PART, C2], bf16, isOutput=True)

    # The two HWDGE rings: SP (sync) and ACT (scalar). A third (SWDGE)
    # stream adds no bandwidth -- the HBM stack is already saturated at
    # ~330 GB/s/core with both stack neighbors streaming.
    def ring(i):
        return nc.sync if i % 2 == 0 else nc.scalar

    with TileContext(nc) as tc:
        with (
            tc.tile_pool(name="const", bufs=1) as constp,
            tc.tile_pool(name="h", bufs=KT) as hp,
            tc.tile_pool(name="psum", bufs=1, space=bass.MemorySpace.PSUM) as pp,
            tc.tile_pool(name="out", bufs=1) as op,
        ):
            # Pre-issue the first four k-tile DMAs BEFORE w: per-ring
            # trigger issue costs ~0.8us each, and the h-stream ramp is
            # trigger-limited for its first ~2us. w can land late -- the PE
            # pre-warm keeps the array busy until well after it arrives.
            h_tiles = {}
            for t in range(6):
                h_tiles[t] = hp.tile([PART, C], bf16, tag="h", name=f"h{t}")
                ring(t).dma_start(h_tiles[t][:], hT[t])
            w_sb = constp.tile([PART, KTM], bf16, tag="w")
            nc.sync.dma_start(w_sb[:], wT[:])
            b_sb = constp.tile([PART, 1], f32, tag="bias")
            nc.gpsimd.dma_start(b_sb[:], bias2[:])

            # PSUM layout: chunks 0/1 stacked two-deep in partitions (PE
            # writes chunk1 at partition offset 64), and SPLIT column-wise
            # over two banks so the final evacuation can run on VectorE and
            # ScalarE in parallel (same-bank parallel PSUM access is
            # serialized by the hardware).
            assert len(offs) == 3 and offs[0][1] == NCH and offs[1][1] == NCH
            SPL = 320
            rem = offs[2][1]
            psA1 = pp.tile([PART, SPL], f32, tag="psA1", name="psA1")
            psA2 = pp.tile([PART, NCH - SPL], f32, tag="psA2", name="psA2")
            psB = pp.tile([MAX_GS, rem], f32, tag="psB", name="psB")

            def targets(n):
                """(psum AP, chunk-relative col range) pairs for chunk n."""
                if n == 0:
                    return [(psA1[0:MAX_GS, :], 0, SPL), (psA2[0:MAX_GS, :], SPL, NCH)]
                if n == 1:
                    return [(psA1[MAX_GS:PART, :], 0, SPL), (psA2[MAX_GS:PART, :], SPL, NCH)]
                return [(psB[:, :], 0, rem)]

            # PE pre-warm: throwaway matmuls on a zeroed tile, issued before
            # any DMA dependency, keep the PE array busy from kernel start
            # so HAM lifts the clock gate (1.2 -> 2.4 GHz). The busy burst
            # must exceed the free-running 3413ns HAM window: 14 x 232ns =
            # 3.2us measurably does NOT fire it; use 24 (~5.6us).
            dw = constp.tile([PART, 256], bf16, tag="dwarm")
            nc.vector.memset(dw[:], 0)
            ps_warm = pp.tile([MAX_GS, 256], f32, tag="pswarm", name="pswarm")
            for _ in range(24):
                nc.tensor.matmul(
                    ps_warm[:, :], dw[:, 0:MAX_GS], dw[:, :],
                    start=True, stop=True,
                )

            # Absorb the w/bias DMA waits into throwaway ops on PE and DVE
            # so each real matmul waits only on its h-tile DMA and the first
            # tensor_scalar_add waits only on PE (one-wait-per-inst limit).
            nc.tensor.matmul(
                ps_warm[:, 0:MAX_GS], w_sb[:, 0:MAX_GS], w_sb[:, 0:MAX_GS],
                start=True, stop=True,
            )
            b_warm = constp.tile([PART, 1], f32, tag="bwarm", name="bwarm")
            nc.vector.tensor_copy(b_warm[:], b_sb[:])

            # k-tiles 0..13: one DMA each, alternating rings. k-tiles 14/15
            # are split column-wise across BOTH rings so the stream does not
            # end with two lonely ~160 GB/s single-ring transfers.
            for t in range(KT - 4):
                if t not in h_tiles:
                    h_tiles[t] = hp.tile([PART, C], bf16, tag="h", name=f"h{t}")
                    ring(t).dma_start(h_tiles[t][:], hT[t])
                h_sb = h_tiles[t]
                for n in range(len(offs)):
                    no = offs[n][0]
                    for ps_ap, c0, c1 in targets(n):
                        nc.tensor.matmul(
                            ps_ap,
                            w_sb[:, t * MAX_GS : (t + 1) * MAX_GS],
                            h_sb[:, no + c0 : no + c1],
                            start=(t == 0),
                            stop=False,
                        )
                if 2 <= t <= KT - 5:
                    # filler matmuls per k-tile keep PE duty high enough
                    # that HAM holds the 2.4 GHz clock through the stream
                    # (at ~55% duty it re-throttles mid-kernel).
                    for _ in range(2 if t <= 9 else 1):
                        nc.tensor.matmul(
                            ps_warm[:, :], dw[:, 0:MAX_GS], dw[:, :],
                            start=True, stop=True,
                        )
            for t in range(KT - 4, KT):
                ha = hp.tile([PART, NCH], bf16, tag=f"ha{t}")
                nc.sync.dma_start(ha[:], hT[t, :, 0:NCH])
                hb = hp.tile([PART, C - NCH], bf16, tag=f"hb{t}")
                nc.scalar.dma_start(hb[:], hT[t, :, NCH:C])
                last = t == KT - 1
                w_ap = w_sb[:, t * MAX_GS : (t + 1) * MAX_GS]
                for ps_ap, c0, c1 in targets(0):
                    nc.tensor.matmul(ps_ap, w_ap, ha[:, c0:c1],
                                     start=False, stop=last)
                for ps_ap, c0, c1 in targets(1):
                    nc.tensor.matmul(ps_ap, w_ap, hb[:, c0:c1],
                                     start=False, stop=last)
                for ps_ap, c0, c1 in targets(2):
                    nc.tensor.matmul(ps_ap, w_ap, hb[:, NCH + c0 : NCH + c1],
                                     start=False, stop=last)

            # ScalarE-side bias absorber, emitted AFTER every scalar-ring
            # trigger so it can't block the h-stream while waiting on the
            # bias DMA.
            b_warm2 = constp.tile([PART, 1], f32, tag="bwarm2", name="bwarm2")
            nc.scalar.copy(b_warm2[:], b_sb[:])

            # Evacuate PSUM in parallel: DVE takes bank psA1, ACT takes
            # banks psA2 + psB. The out DMA is issued by the scalar engine,
            # so the ACT writes are ordered before it implicitly and its
            # single semaphore wait covers the DVE write.
            o_sb = op.tile([PART, C2], bf16, tag="o")
            nc.vector.memset(o_sb[MAX_GS:PART, NCH:C2], 0)
            nc.vector.tensor_scalar_add(o_sb[:, 0:SPL], psA1[:, :], b_sb[:])
            nc.scalar.add(o_sb[:, SPL:NCH], psA2[:, :], b_sb[:])
            nc.scalar.add(o_sb[0:MAX_GS, NCH:C2], psB[:, :], b_sb[0:MAX_GS])
            out_dma = nc.scalar.dma_start(outT[:], o_sb[:])

    # The out DMA reuses a DMAHW lane, so Tile gives it a lane-reuse wait
    # on top of its DVE data wait -- two waits, which this walrus rejects.
    # The lane's previous DMA is an h-tile transfer that every matmul (and
    # hence the DVE adds) already transitively ordered before us, so the
    # lane wait is provably satisfied: strip it, keep the DVE wait.
    si = out_dma.ins.sync_info
    if si is not None and len(si.on_wait) > 1:
        keep = [w for w in si.on_wait if "DVE" in (w.ant_name or "")]
        assert len(keep) == 1, [str(w) for w in si.on_wait]
        si.on_wait = keep
        out_dma.ins.sync_info = si

    # Reorder the kernel-tail drain chain so the single drain that waits on
    # the out DMA's completion sem runs LAST: the other lane waits are
    # satisfied mid-stream, and executing them first means the sync engine
    # takes its one slow semaphore wake-up on the final drain only.
    out_sem = out_dma.ins.sync_info.on_update[0].id
    drains = []
    for f in nc.m.functions:
        for blk in f.blocks:
            for inst in blk.instructions:
                if type(inst).__name__ == "InstDrain":
                    si2 = inst.sync_info
                    if si2 is not None and len(si2.on_wait) == 1:
                        drains.append(inst)
    hit = [d for d in drains if d.sync_info.on_wait[0].id == out_sem]
    if hit and drains and hit[0] is not drains[-1]:
        d_out, d_last = hit[0], drains[-1]
        si_a, si_b = d_out.sync_info, d_last.sync_info
        d_out.sync_info, d_last.sync_info = si_b, si_a

    return nc


def kernel(**inputs):
    global LAST_EXEC_NS, LAST_RESULTS
    from concourse.bass_utils import run_bass_kernel_spmd

    hidden = np.ascontiguousarray(np.asarray(inputs["hidden"], dtype=np.float32))
    idx = np.asarray(inputs["chosen_group_idx"]).astype(np.int64)
    W = np.asarray(inputs["W"], dtype=np.float32)
    b = np.asarray(inputs["b"], dtype=np.float32)
    gs = np.asarray(inputs["group_sizes"])

    B = hidden.shape[0]
    counts = np.bincount(idx, minlength=N_GROUPS)
    C = max(1040, int(-(-counts.max() // 16)) * 16)
    offs, C2 = _chunks(C)

    positions = [np.nonzero(idx == g)[0] for g in range(N_GROUPS)]

    bf = ml_dtypes.bfloat16
    in_maps = []
    for g in range(N_GROUPS):
        pos = positions[g]
        hg = np.zeros((C, D_MODEL), bf)
        hg[: len(pos)] = hidden[pos, g, :].astype(bf)
        hT = np.ascontiguousarray(hg.T).reshape(KT, PART, C)
        wT = np.ascontiguousarray(
            W[g].reshape(MAX_GS, KT, PART).transpose(2, 1, 0)
        ).reshape(PART, KT * MAX_GS).astype(bf)
        bias2 = np.ascontiguousarray(
            np.concatenate([b[g], b[g]])[:, None].astype(np.float32)
        )
        in_maps.append({"hT": hT, "wT": wT, "bias2": bias2})

    if C not in _nc_cache:
        _nc_cache[C] = _build_nc(C)
    nc = _nc_cache[C]

    res = run_bass_kernel_spmd(nc, in_maps, list(range(N_GROUPS)), trace=TRACE)
    LAST_EXEC_NS = res.exec_time_ns
    LAST_RESULTS = res

    preds = np.zeros((B, MAX_GS), np.float32)
    for g in range(N_GROUPS):
        pos = positions[g]
        outT = np.asarray(res.results[g]["outT"]).astype(np.float32)  # [128, C2]
        og = np.zeros((C, MAX_GS), np.float32)
        col = 0
        for i in range(0, len(offs), 2):
            no, ns = offs[i]
            og[no : no + ns] = outT[0:MAX_GS, col : col + ns].T
            if i + 1 < len(offs):
                no1, ns1 = offs[i + 1]
                og[no1 : no1 + ns1] = outT[MAX_GS:PART, col : col + ns1].T
            col += ns
        preds[pos] = og[: len(pos)]

    valid = np.arange(MAX_GS)[None, :] < gs[idx][:, None]
    preds = np.where(valid, preds, np.float32(0.0))
    return preds, valid
